# revision 74
# baseline (speedup 1.0000x reference)
"""Trainium2 Bass kernel for causal multi-head attention with RoPE.

Problem: B=4, S=2048, D=768, H=12, HD=64 (torch-Linear style projections,
rotary embeddings on q/k, causal softmax, output projection + bias).

Sharding across 8 NeuronCores: core c handles batch c//2 and head-group
c%2 (6 of 12 heads). Each core computes a partial output projection
(its heads' contribution to ctx @ Wo.T); the host sums the two partials
per batch and adds the bias. No device collectives.

Per-core kernel (matmul operands bf16, fp32 PSUM accumulation):
  - Q^T/K^T [128, S] per head-pair (two heads stacked 64+64 on the
    partition dim), projected in 512-col chunks drip-fed between
    attention steps. RoPE: t = ps*cos and u2 = ps*sinN (source-signed
    sin) on DVE; rotate_half is the partition permutation p ^ 32 — four
    [32, 1024] SBUF->SBUF DMAs plus a GpSimd add in steady state, or a
    PE permutation matmul during the eager prologue (PE idles there
    while the SP queue streams eT).
  - V [S, hd] per head with an appended ones column (row 64 of the PV
    accumulator becomes the softmax denominator for free). V shares its
    chunk's eT bytes so PE work per DMA byte beats the stream rate.
  - Attention in 512-query windows (the last one as two 256-col halves
    so its output chunks drain during attention), per head-pair: the two
    heads' score matmuls are row-tiled (K=64 stationaries at PE row
    groups 0-1 / 2-3) so they can run concurrently. One exp covers both
    heads' PSUM banks via a [128, 2, w] access pattern (scale=1/8 folded
    in, no max subtraction; scores are bounded); exp output is bf16; the
    causal diag mask is a 0/1 multiply on DVE after the exp. PV runs one
    k-chunk behind the exp stream so PE's in-order queue never stalls.
  - Denominator row is DMA-broadcast to 64 partitions, reciprocal'd, and
    multiplied into the evicted ctx tile.
  - out = ctx^T-chunks.T @ Wo^T-chunks, drip-fed into the pair-2
    attention windows; partial output summed on host.
"""

import numpy as np

B, S, D, H = 4, 2048, 768, 12
HD = D // H          # 64
N_CORES = 8
HEADS_PER_CORE = 6
PAIRS = 3            # head pairs per core
DC = D // 128        # 6 contraction chunks
NJ = S // 128        # 16 k-chunks
W = 512              # q-window width
NW = S // W          # 4 windows
HW_ = 1024           # projection half-pair width

_CACHE = {}


def _rope_tables():
    inv_freq = 1.0 / (10000.0 ** (np.arange(0, HD, 2, dtype=np.float64) / HD))
    ang = np.arange(S, dtype=np.float64)[:, None] * inv_freq[None, :]  # [S, 32]
    cos = np.cos(ang).astype(np.float32)   # [S, 32]
    sin = np.sin(ang).astype(np.float32)
    cosF = np.empty((128, S), np.float32)
    sinN = np.empty((128, S), np.float32)
    for g in range(4):
        cosF[32 * g:32 * g + 32] = cos.T
        # source-signed: sinN[k] = -sgn(k) * sin, so that after the p^32
        # partition permutation, ur[p] = sgn(p) * sin[p] * q[p^32]
        sgn = 1.0 if g % 2 == 0 else -1.0
        sinN[32 * g:32 * g + 32] = sgn * sin.T
    return cosF, sinN


def _build_program(reps=1, dbg=False, ablate=()):
    import concourse.bacc as bacc
    import concourse.mybir as mybir
    import concourse.tile as tile

    f32 = mybir.dt.float32
    bf16 = mybir.dt.bfloat16
    f8 = mybir.dt.float8e4
    DR = mybir.MatmulPerfMode.DoubleRow
    AF = mybir.ActivationFunctionType
    OP = mybir.AluOpType

    nc = bacc.Bacc("TRN2", target_bir_lowering=False, debug=False,
                   num_devices=N_CORES)

    eT = nc.declare_dram_parameter("eT", [D, S], bf16, isOutput=False)
    wq = nc.declare_dram_parameter("wq", [D, 384], bf16, isOutput=False)
    wk = nc.declare_dram_parameter("wk", [D, 384], bf16, isOutput=False)
    wv = nc.declare_dram_parameter("wv", [D, 384], bf16, isOutput=False)
    wo = nc.declare_dram_parameter("wo", [384, D], bf16, isOutput=False)
    cosF_d = nc.declare_dram_parameter("cosF", [128, S], bf16, isOutput=False)
    sinN_d = nc.declare_dram_parameter("sinN", [128, S], bf16, isOutput=False)
    trm_d = nc.declare_dram_parameter("trm", [128, 2, 128], bf16,
                                      isOutput=False)
    prm_d = nc.declare_dram_parameter("prm", [128, 128], bf16,
                                      isOutput=False)
    o = nc.declare_dram_parameter("o", [S, D], f32, isOutput=True)
    if dbg:
        qtd = nc.declare_dram_parameter("qtd", [128, PAIRS, S], bf16,
                                        isOutput=True)
        ktd = nc.declare_dram_parameter("ktd", [128, PAIRS, S], bf16,
                                        isOutput=True)
        vtd = nc.declare_dram_parameter("vtd", [128, NJ, HEADS_PER_CORE,
                                                HD + 1], bf16, isOutput=True)
        cxtd = nc.declare_dram_parameter("cxtd", [128, PAIRS, S], bf16,
                                         isOutput=True)

    with tile.TileContext(nc) as tc, \
            nc.allow_low_precision(reason="bf16 matmul operand tiles"):
        with tc.tile_pool(name="const", bufs=1) as cp:
            cosF = cp.tile([128, S], bf16)
            sinN = cp.tile([128, S], bf16)
            trm = cp.tile([128, 2, 128], bf16)
            prm = cp.tile([128, 128], bf16)

            qt = cp.tile([128, PAIRS, S], bf16)
            kt = cp.tile([128, PAIRS, S], bf16)
            vt = cp.tile([128, NJ, HEADS_PER_CORE, HD + 1], bf16)
            nc.vector.memset(vt[:, :, :, HD].bitcast(mybir.dt.uint16),
                             0x3F80)
            cxt = cp.tile([128, PAIRS, S], bf16)
            if "noexp" in ablate:
                etc_ = cp.tile([128, 2, 2, W], bf16)
                nc.vector.memset(etc_[:].bitcast(mybir.dt.uint16), 0x3F00)
            if "noattn" in ablate:
                nc.vector.memset(cxt[:].bitcast(mybir.dt.uint16), 0x3F00)
            if "noproj" in ablate:
                nc.vector.memset(qt[:].bitcast(mybir.dt.uint16), 0x3F00)
                nc.vector.memset(kt[:].bitcast(mybir.dt.uint16), 0x3F00)
            wot = cp.tile([128, PAIRS, D], bf16)

            eT_r = eT[:].rearrange("(n p) s -> p n s", p=128)

            for _rep in range(reps):
                with (
                    tc.tile_pool(name="asb", bufs=3) as asb,
                    tc.tile_pool(name="projsb", bufs=1) as pjs,
                ):
                    scp_cm = tc.tile_pool(name="scp", bufs=2, space="PSUM")
                    scp = scp_cm.__enter__()
                    cxp_cm = tc.tile_pool(name="cxp", bufs=2, space="PSUM")
                    cxp = cxp_cm.__enter__()
                    pps_cm = tc.tile_pool(name="pps", bufs=2, space="PSUM")
                    pps = pps_cm.__enter__()

                    _DONE = object()
                    et_cache, rope_state = {}, {}

                    def consume(filler, n):
                        if filler is None:
                            return
                        for _ in range(n):
                            if next(filler, _DONE) is _DONE:
                                return

                    def proj_chunk(pair, cc, with_v, wqt, wkt, wvt,
                                   eager=False):
                        """One 512-col chunk of Q^T/K^T (+V when with_v) as
                        a generator of small PE quanta. RoPE state (t, u2)
                        accumulates across a 1024-col chunk pair; the
                        rotate-half DMAs and adds fire when the odd chunk
                        (or an eager chunk) completes."""
                        if "noproj" in ablate:
                            return
                        hh, sub = cc // 2, cc % 2
                        etk = (pair, hh)
                        if sub == 0:
                            # the eT half depends only on the column range,
                            # not the head-pair: download each 1.57MB half
                            # once and share the tile across all three
                            # pairs' projections
                            if hh not in et_cache:
                                etAB = pjs.tile([128, 6, HW_], bf16,
                                                tag=f"et{hh}", bufs=1,
                                                name=f"eA{hh}")
                                et_cache[hh] = etAB
                                c0 = HW_ * hh
                                if eager:
                                    for d in range(6):
                                        nc.sync.dma_start(
                                            etAB[:, d:d + 1, 0:512],
                                            eT_r[:, d:d + 1, c0:c0 + 512])
                                    nc.sync.dma_start(
                                        etAB[:, :, 512:HW_],
                                        eT_r[:, :, c0 + 512:c0 + HW_])
                                else:
                                    for sp in range(3):
                                        dd = slice(2 * sp, 2 * sp + 2)
                                        eng = (nc.sync if sp % 2 == 0
                                               else nc.scalar)
                                        eng.dma_start(etAB[:, dd, :],
                                                      eT_r[:, dd,
                                                           c0:c0 + HW_])
                            rope_state[etk] = {}
                            for wx in (0, 1):
                                rope_state[etk][wx] = (
                                    pjs.tile([128, HW_], bf16, tag=f"t{wx}",
                                             bufs=2, name=f"t{wx}{pair}{hh}"),
                                    pjs.tile([128, HW_], bf16, tag=f"u{wx}",
                                             bufs=2, name=f"u{wx}{pair}{hh}"),
                                    pjs.tile([128, HW_], bf16, tag=f"r{wx}",
                                             bufs=2, name=f"r{wx}{pair}{hh}"))
                        etAB = et_cache[hh]

                        def rope(wx, dst, ss, cols):
                            t_t, u2, ur = rope_state[etk][wx]
                            if eager:
                                # prologue: PE is idle and the SP queue is
                                # busy streaming eT — rotate via a PE
                                # permutation matmul instead of DMAs
                                ps2 = pps.tile([128, 512], f32, tag="ps",
                                               name=f"pr{wx}{pair}{cc}")
                                nc.tensor.matmul(ps2[:], prm[:], u2[:, ss],
                                                 start=True, stop=True)
                                nc.vector.tensor_tensor(
                                    dst[:, pair, cols], ps2[:], t_t[:, ss],
                                    OP.add)
                                return
                            for g in range(4):
                                d0, s0 = 32 * g, 32 * (g ^ 1)
                                nc.sync.dma_start(ur[d0:d0 + 32, ss],
                                                  u2[s0:s0 + 32, ss])
                            nc.gpsimd.tensor_tensor(
                                dst[:, pair, cols], t_t[:, ss],
                                ur[:, ss], OP.add)

                        cols = slice(512 * cc, 512 * cc + 512)
                        ss = slice(512 * sub, 512 * sub + 512)
                        pend_rope = []
                        for wx, wt, dst in ((0, wqt, qt), (1, wkt, kt)):
                            t_t, u2, ur = rope_state[etk][wx]
                            ps = pps.tile([128, 512], f32, tag="ps",
                                          name=f"ps{wx}{pair}{cc}")
                            for d in range(DC):
                                nc.tensor.matmul(
                                    ps[:],
                                    wt[:, d, 128 * pair:128 * pair + 128],
                                    etAB[:, d, 512 * sub:512 * sub + 512],
                                    start=(d == 0), stop=(d == DC - 1))
                                if d % 2 == 1:
                                    yield
                            nc.vector.tensor_tensor(
                                u2[:, ss], ps[:], sinN[:, cols], OP.mult)
                            nc.vector.tensor_tensor(
                                t_t[:, ss], ps[:], cosF[:, cols], OP.mult)
                            if eager:
                                # defer the perm matmul past the V chunks so
                                # its DVE dependency doesn't block the
                                # in-order PE queue
                                pend_rope.append((wx, dst, ss, cols))
                            yield
                        if sub == 1 and not eager:
                            hc = slice(HW_ * hh, HW_ * hh + HW_)
                            hs = slice(0, HW_)
                            rope(0, qt, hs, hc)
                            rope(1, kt, hs, hc)
                            yield
                        if with_v:
                            # V reuses the just-streamed eT bytes — keep it
                            # with its chunk so PE work per DMA byte stays
                            # above the stream rate
                            yield from v_chunks(4 * cc, 4 * cc + 4, etAB)
                        for args in pend_rope:
                            rope(*args)
                        if pend_rope:
                            yield

                    def v_chunks(lo, hi, etAB):
                        for i in range(lo, hi):
                            io = 128 * (i % 8)
                            sub, ioo = io // 512, io % 512
                            pv = pps.tile([128, 384], f32, tag="ps",
                                          name=f"pv{i}")
                            for d in range(DC):
                                nc.tensor.matmul(
                                    pv[:],
                                    etAB[:, d, 512 * sub + ioo:
                                         512 * sub + ioo + 128],
                                    wvt[:, d, :],
                                    start=(d == 0), stop=(d == DC - 1))
                                if d % 2 == 1 and d < 5:
                                    yield
                            # ACT is idle early in the kernel (few exps
                            # yet) while DVE carries the RoPE chain — evict
                            # V on ACT
                            nc.scalar.activation(vt[:, i, :, 0:HD], pv[:],
                                                 AF.Copy)
                            yield

                    def attn_win(pair, base, width, filler=None, warmup=0,
                                 evict_act=False):
                        """Attention for both heads of `pair` on the q-window
                        [base, base+width). `warmup` delays filler
                        consumption by that many k-chunks so fillers whose
                        inputs come from the previous window's eviction
                        don't stall the in-order PE queue."""
                        if "noattn" in ablate:
                            consume(filler, 1000)
                            return
                        nj = (base + width) // 128
                        CA = cxp.tile([HD + 1, W], f32, tag="C",
                                      name=f"CA{pair}{base}")
                        CB = cxp.tile([HD + 1, W], f32, tag="C",
                                      name=f"CB{pair}{base}")

                        nfull = base // 128

                        def emit_pv(j, et_, qlo):
                            off = qlo - base
                            wj = width - off
                            stop = j == nj - 1
                            for hx, C in ((0, CA), (1, CB)):
                                nc.tensor.matmul(
                                    C[:, off:off + wj],
                                    vt[:, j, 2 * pair + hx, :],
                                    et_[:, hx, 0:wj],
                                    start=(j == 0), stop=stop)

                        pend = None
                        for j in range(nj):
                            qlo = max(base, 128 * j)
                            wj = base + width - qlo
                            kk = slice(128 * j, 128 * j + 128)
                            sc = scp.tile([128, 2, W], f32, tag="sc",
                                          name=f"sc{pair}{base}{j}")
                            diag = j >= nfull
                            # two heads' score matmuls are row-tiled (K=64
                            # stationaries at partition bases 0/64) so they
                            # occupy disjoint PE row groups and overlap
                            for hx in (0, 1):
                                b0 = 64 * hx
                                nc.tensor.matmul(
                                    sc[:, hx, 0:wj],
                                    kt[b0:b0 + 64, pair, kk],
                                    qt[b0:b0 + 64, pair, qlo:qlo + wj],
                                    start=True, stop=True)
                            if "noexp" in ablate:
                                et_ = etc_
                            else:
                                et_ = asb.tile([128, 2, W], bf16, tag="ex",
                                               bufs=3,
                                               name=f"ex{pair}{base}{j}")
                                nc.scalar.activation(
                                    et_[:, :, 0:wj], sc[:, :, 0:wj],
                                    AF.Exp, scale=0.125)
                                if diag:
                                    # zero the upper-triangle weights (the
                                    # causal mask) on DVE, off the critical
                                    # sc->exp path (PV runs one j behind)
                                    nc.vector.tensor_tensor(
                                        et_[:, :, 0:128], et_[:, :, 0:128],
                                        trm[:], OP.mult)
                            # software pipeline: PV runs one j behind so
                            # PE's in-order queue never waits on exp_j
                            if pend is not None:
                                emit_pv(*pend)
                            pend = (j, et_, qlo)
                            # slip quanta of projection/output work into
                            # the PE queue behind this step's matmuls
                            # (narrow windows have tiny attn matmuls and
                            # need a faster feed to stay busy)
                            if j >= warmup:
                                consume(filler, 2 if width <= 256 else 1)
                        emit_pv(*pend)

                        cs = slice(base, base + width)
                        rec = asb.tile([128, W], bf16, tag="rec", bufs=2,
                                       name=f"rc{pair}{base}")
                        for hx, C in ((0, CA), (1, CB)):
                            po = HD * hx
                            # evict first (frees the PSUM C ring slot
                            # fast), then scale by 1/denom in place. At the
                            # tail ACT does the copy (no exps left there).
                            if evict_act:
                                nc.scalar.activation(
                                    cxt[po:po + HD, pair, cs],
                                    C[0:HD, 0:width], AF.Copy)
                            else:
                                nc.vector.tensor_copy(
                                    cxt[po:po + HD, pair, cs],
                                    C[0:HD, 0:width])
                            rr = asb.tile([1, W], bf16, tag="rr", bufs=2,
                                          name=f"rr{pair}{base}{hx}")
                            nc.vector.reciprocal(rr[0:1, 0:width],
                                                 C[HD:HD + 1, 0:width])
                            nc.sync.dma_start(
                                rec[po:po + HD, 0:width],
                                rr[0:1, None, 0:width].to_broadcast(
                                    [1, HD, width]))
                            nc.vector.tensor_tensor(
                                cxt[po:po + HD, pair, cs],
                                cxt[po:po + HD, pair, cs],
                                rec[po:po + HD, 0:width], OP.mult)
                        consume(filler, 1000)   # drain

                    def out_emit(i, osp):
                        op_ = osp.tile([128, D], f32, tag="op", name=f"op{i}")
                        ss = slice(128 * i, 128 * i + 128)
                        for pair in range(PAIRS):
                            for c0 in range(0, D, 512):
                                cw = min(512, D - c0)
                                nc.tensor.matmul(
                                    op_[:, c0:c0 + cw],
                                    cxt[:, pair, ss],
                                    wot[:, pair, c0:c0 + cw],
                                    start=(pair == 0),
                                    stop=(pair == PAIRS - 1))
                        ot = asb.tile([128, D], f32, tag="ot", bufs=3,
                                      name=f"ot{i}")
                        # tail: evict halves on ACT and DVE in parallel,
                        # store halves on both DMA queues
                        nc.scalar.activation(ot[:, 0:384], op_[:, 0:384],
                                             AF.Copy)
                        nc.vector.tensor_copy(ot[:, 384:D], op_[:, 384:D])
                        nc.sync.dma_start(o[ss, 0:384], ot[:, 0:384])
                        nc.scalar.dma_start(o[ss, 384:D], ot[:, 384:D])

                    def out_gen(lo, hi, osp):
                        for i in range(lo, hi):
                            op_ = osp.tile([128, D], f32, tag="op",
                                           name=f"op{i}")
                            ss = slice(128 * i, 128 * i + 128)
                            for pair in range(PAIRS):
                                for c0 in range(0, D, 512):
                                    cw = min(512, D - c0)
                                    nc.tensor.matmul(
                                        op_[:, c0:c0 + cw],
                                        cxt[:, pair, ss],
                                        wot[:, pair, c0:c0 + cw],
                                        start=(pair == 0),
                                        stop=(pair == PAIRS - 1))
                                yield
                            ot = asb.tile([128, D], f32, tag="ot", bufs=3,
                                          name=f"ot{i}")
                            # DVE copy: its queue is shorter than ACT's exp
                            # backlog, so the PSUM ring slot frees sooner
                            nc.vector.tensor_copy(ot[:], op_[:])
                            eng = nc.sync if i % 2 == 0 else nc.scalar
                            eng.dma_start(o[ss, :], ot[:])
                            yield

                    # weights/tables on the ScalarE DMA queue so the eT
                    # stream (SP queue) starts immediately. Pair-0 weight
                    # slices and the first cos/sin half come first — they
                    # gate the eager projection and window 0.
                    wq_r = wq[:].rearrange("(n p) m -> p n m", p=128)
                    wk_r = wk[:].rearrange("(n p) m -> p n m", p=128)
                    wqt = pjs.tile([128, DC, 384], bf16)
                    for dd0 in range(0, DC, 2):
                        nc.scalar.dma_start(
                            wqt[:, dd0:dd0 + 2, 0:128],
                            wq_r[:, dd0:dd0 + 2, 0:128])
                    wkt = pjs.tile([128, DC, 384], bf16)
                    nc.scalar.dma_start(wkt[:, :, 0:128], wk_r[:, :, 0:128])
                    nc.scalar.dma_start(cosF[:, 0:512], cosF_d[:, 0:512])
                    nc.scalar.dma_start(sinN[:, 0:512], sinN_d[:, 0:512])
                    wvt = pjs.tile([128, DC, 384], bf16)
                    nc.scalar.dma_start(
                        wvt[:], wv[:].rearrange("(n p) m -> p n m", p=128))
                    nc.scalar.dma_start(trm[:], trm_d[:])
                    nc.scalar.dma_start(prm[:], prm_d[:])
                    nc.scalar.dma_start(wqt[:, :, 128:384],
                                        wq_r[:, :, 128:384])
                    nc.scalar.dma_start(wkt[:, :, 128:384],
                                        wk_r[:, :, 128:384])
                    nc.scalar.dma_start(cosF[:, 512:S], cosF_d[:, 512:S])
                    nc.scalar.dma_start(sinN[:, 512:S], sinN_d[:, 512:S])
                    # wot is not needed until the output projection — keep it
                    # off the SP queue so the first eT chunk lands immediately
                    nc.scalar.dma_start(
                        wot[:], wo[:].rearrange("(n p) m -> p n m", p=128))

                    # pipeline: attention windows are the backbone;
                    # projection and output-projection matmuls are drip-fed
                    # between attention steps. win(p, w) needs pair-p halves
                    # 0..(w>=2) only; each g(p, h) covers 1024 q-columns.
                    from itertools import chain as _chain

                    def g(pair, cc, eager=False):
                        return proj_chunk(pair, cc, pair == 0,
                                          wqt, wkt, wvt, eager=eager)

                    # warm-up: throwaway matmuls against the first
                    # weight slice while the eT stream lands, so the PE
                    # clock is ramped (and HW HAM un-throttled) before the
                    # first real projection matmuls issue
                    warm = pps.tile([128, 128], f32, tag="ps", name="warm")
                    for _ in range(24):
                        nc.tensor.matmul(warm[:], wqt[:, 0, 0:128],
                                         wqt[:, 0, 0:128],
                                         start=True, stop=True)
                    consume(g(0, 0, eager=True), 1000)
                    consume(g(0, 1, eager=True), 1000)  # feeds win(0, 0..1)
                    attn_win(0, 0, W, g(0, 2))
                    attn_win(0, W, W, g(0, 3))
                    attn_win(0, 2 * W, W, g(1, 0))
                    attn_win(0, 3 * W, W, g(1, 1))
                    attn_win(1, 0, W, g(1, 2))
                    attn_win(1, W, W, g(1, 3))
                    attn_win(1, 2 * W, W, g(2, 0))
                    attn_win(1, 3 * W, W, g(2, 1))
                    attn_win(2, 0, W, _chain(g(2, 2), g(2, 3)))
                    # projection PSUM banks are dead now — recycle for the
                    # output projection so it overlaps pair-2 attention
                    pps_cm.__exit__(None, None, None)
                    osp_cm = tc.tile_pool(name="osp", bufs=1, space="PSUM")
                    osp = osp_cm.__enter__()
                    attn_win(2, W, W, out_gen(0, 4, osp), warmup=2)
                    attn_win(2, 2 * W, W, out_gen(4, 8, osp), warmup=2)
                    # the last 512-query window runs as two 256-col halves
                    # so its output chunks drain during attention instead
                    # of in a serial tail
                    # last 512-query window as 384 + 128 halves: the
                    # final 128-col window leaves only out-chunk 15 as a
                    # serial tail, and its spilled out matmuls overlap the
                    # final eviction chain on the other engines
                    attn_win(2, 3 * W, 384, out_gen(8, 10, osp),
                             warmup=2, evict_act=True)
                    attn_win(2, 3 * W + 384, 128,
                             _chain(out_gen(10, 12, osp),
                                    out_gen(12, 15, osp)),
                             warmup=2, evict_act=True)
                    osp_cm.__exit__(None, None, None)
                    cxp_cm.__exit__(None, None, None)
                    scp_cm.__exit__(None, None, None)
                    # tail: final chunk with a deep PSUM ring
                    osp2_cm = tc.tile_pool(name="osp2", bufs=3, space="PSUM")
                    osp2 = osp2_cm.__enter__()
                    out_emit(15, osp2)
                    osp2_cm.__exit__(None, None, None)
                    if dbg:
                        nc.sync.dma_start(qtd[:], qt[:])
                        nc.sync.dma_start(ktd[:], kt[:])
                        nc.sync.dma_start(vtd[:], vt[:])
                        nc.sync.dma_start(cxtd[:], cxt[:])

    nc.compile()
    return nc


def _get_program(reps=1, ablate=()):
    key = (reps, tuple(ablate))
    if key not in _CACHE:
        _CACHE[key] = _build_program(reps, ablate=ablate)
    return _CACHE[key]


def make_in_maps(embeds, Wq, Wk, Wv, Wo):
    import ml_dtypes
    bf16 = ml_dtypes.bfloat16
    f8 = ml_dtypes.float8_e4m3
    cosF, sinN = _rope_tables()
    cosF, sinN = cosF.astype(bf16), sinN.astype(bf16)
    prm = np.zeros((128, 128), np.float32)
    prm[np.arange(128), np.arange(128) ^ 32] = 1.0
    prm = prm.astype(bf16)
    trm1 = (np.arange(128)[None, :] >= np.arange(128)[:, None])
    trm = np.ascontiguousarray(
        np.broadcast_to(trm1[:, None, :], (128, 2, 128))).astype(bf16)
    eTs = [np.ascontiguousarray(embeds[b].T).astype(bf16) for b in range(B)]
    in_maps = []
    for c in range(N_CORES):
        b, hg = c // 2, c % 2
        hs = slice(hg * 384, hg * 384 + 384)
        in_maps.append({
            "eT": eTs[b],
            "wq": np.ascontiguousarray(Wq[hs].T).astype(bf16),
            "wk": np.ascontiguousarray(Wk[hs].T).astype(bf16),
            "wv": np.ascontiguousarray(Wv[hs].T).astype(bf16),
            "wo": np.ascontiguousarray(Wo[:, hs].T).astype(bf16),
            "cosF": cosF, "sinN": sinN, "trm": trm, "prm": prm,
        })
    return in_maps


def kernel(embeds, Wq, Wk, Wv, Wo, bo):
    from concourse.bass_utils import run_bass_kernel_spmd

    embeds = np.asarray(embeds, np.float32)
    Wq = np.asarray(Wq, np.float32)
    Wk = np.asarray(Wk, np.float32)
    Wv = np.asarray(Wv, np.float32)
    Wo = np.asarray(Wo, np.float32)
    bo = np.asarray(bo, np.float32)

    nc = _get_program()
    in_maps = make_in_maps(embeds, Wq, Wk, Wv, Wo)
    res = run_bass_kernel_spmd(nc, in_maps, list(range(N_CORES))).results
    out = np.empty((B, S, D), np.float32)
    for b in range(B):
        out[b] = res[2 * b]["o"] + res[2 * b + 1]["o"] + bo
    return out


# revision 77
# speedup vs baseline: 1.0117x; 1.0117x over previous
"""Trainium2 Bass kernel for causal multi-head attention with RoPE.

Problem: B=4, S=2048, D=768, H=12, HD=64 (torch-Linear style projections,
rotary embeddings on q/k, causal softmax, output projection + bias).

Sharding across 8 NeuronCores: core c handles batch c//2 and head-group
c%2 (6 of 12 heads). Each core computes a partial output projection
(its heads' contribution to ctx @ Wo.T); the host sums the two partials
per batch and adds the bias. No device collectives.

Per-core kernel (matmul operands bf16, fp32 PSUM accumulation):
  - Q^T/K^T [128, S] per head-pair (two heads stacked 64+64 on the
    partition dim), projected in 512-col chunks drip-fed between
    attention steps. RoPE: t = ps*cos and u2 = ps*sinN (source-signed
    sin) on DVE; rotate_half is the partition permutation p ^ 32 — four
    [32, 1024] SBUF->SBUF DMAs plus a GpSimd add in steady state, or a
    PE permutation matmul during the eager prologue (PE idles there
    while the SP queue streams eT).
  - V [S, hd] per head with an appended ones column (row 64 of the PV
    accumulator becomes the softmax denominator for free). V shares its
    chunk's eT bytes so PE work per DMA byte beats the stream rate.
  - Attention in 512-query windows (the last one as two 256-col halves
    so its output chunks drain during attention), per head-pair: the two
    heads' score matmuls are row-tiled (K=64 stationaries at PE row
    groups 0-1 / 2-3) so they can run concurrently. One exp covers both
    heads' PSUM banks via a [128, 2, w] access pattern (scale=1/8 folded
    in, no max subtraction; scores are bounded); exp output is bf16; the
    causal diag mask is a 0/1 multiply on DVE after the exp. PV runs one
    k-chunk behind the exp stream so PE's in-order queue never stalls.
  - Denominator row is DMA-broadcast to 64 partitions, reciprocal'd, and
    multiplied into the evicted ctx tile.
  - out = ctx^T-chunks.T @ Wo^T-chunks, drip-fed into the pair-2
    attention windows; partial output summed on host.
"""

import numpy as np

B, S, D, H = 4, 2048, 768, 12
HD = D // H          # 64
N_CORES = 8
HEADS_PER_CORE = 6
PAIRS = 3            # head pairs per core
DC = D // 128        # 6 contraction chunks
NJ = S // 128        # 16 k-chunks
W = 512              # q-window width
NW = S // W          # 4 windows
HW_ = 1024           # projection half-pair width

_CACHE = {}


def _rope_tables():
    inv_freq = 1.0 / (10000.0 ** (np.arange(0, HD, 2, dtype=np.float64) / HD))
    ang = np.arange(S, dtype=np.float64)[:, None] * inv_freq[None, :]  # [S, 32]
    cos = np.cos(ang).astype(np.float32)   # [S, 32]
    sin = np.sin(ang).astype(np.float32)
    cosF = np.empty((128, S), np.float32)
    sinN = np.empty((128, S), np.float32)
    for g in range(4):
        cosF[32 * g:32 * g + 32] = cos.T
        # source-signed: sinN[k] = -sgn(k) * sin, so that after the p^32
        # partition permutation, ur[p] = sgn(p) * sin[p] * q[p^32]
        sgn = 1.0 if g % 2 == 0 else -1.0
        sinN[32 * g:32 * g + 32] = sgn * sin.T
    return cosF, sinN


def _build_program(reps=1, dbg=False, ablate=()):
    import concourse.bacc as bacc
    import concourse.mybir as mybir
    import concourse.tile as tile

    f32 = mybir.dt.float32
    bf16 = mybir.dt.bfloat16
    f8 = mybir.dt.float8e4
    DR = mybir.MatmulPerfMode.DoubleRow
    AF = mybir.ActivationFunctionType
    OP = mybir.AluOpType

    nc = bacc.Bacc("TRN2", target_bir_lowering=False, debug=False,
                   num_devices=N_CORES)

    eT = nc.declare_dram_parameter("eT", [D, S], bf16, isOutput=False)
    wq = nc.declare_dram_parameter("wq", [D, 384], bf16, isOutput=False)
    wk = nc.declare_dram_parameter("wk", [D, 384], bf16, isOutput=False)
    wv = nc.declare_dram_parameter("wv", [D, 384], bf16, isOutput=False)
    wo = nc.declare_dram_parameter("wo", [384, D], bf16, isOutput=False)
    cosF_d = nc.declare_dram_parameter("cosF", [128, S], bf16, isOutput=False)
    sinN_d = nc.declare_dram_parameter("sinN", [128, S], bf16, isOutput=False)
    trm_d = nc.declare_dram_parameter("trm", [128, 2, 128], bf16,
                                      isOutput=False)
    prm_d = nc.declare_dram_parameter("prm", [128, 128], bf16,
                                      isOutput=False)
    o = nc.declare_dram_parameter("o", [S, D], bf16, isOutput=True)
    if dbg:
        qtd = nc.declare_dram_parameter("qtd", [128, PAIRS, S], bf16,
                                        isOutput=True)
        ktd = nc.declare_dram_parameter("ktd", [128, PAIRS, S], bf16,
                                        isOutput=True)
        vtd = nc.declare_dram_parameter("vtd", [128, NJ, HEADS_PER_CORE,
                                                HD + 1], bf16, isOutput=True)
        cxtd = nc.declare_dram_parameter("cxtd", [128, PAIRS, S], bf16,
                                         isOutput=True)

    with tile.TileContext(nc) as tc, \
            nc.allow_low_precision(reason="bf16 matmul operand tiles"):
        with tc.tile_pool(name="const", bufs=1) as cp:
            cosF = cp.tile([128, S], bf16)
            sinN = cp.tile([128, S], bf16)
            trm = cp.tile([128, 2, 128], bf16)
            prm = cp.tile([128, 128], bf16)

            qt = cp.tile([128, PAIRS, S], bf16)
            kt = cp.tile([128, PAIRS, S], bf16)
            vt = cp.tile([128, NJ, HEADS_PER_CORE, HD + 1], bf16)
            nc.vector.memset(vt[:, :, :, HD].bitcast(mybir.dt.uint16),
                             0x3F80)
            cxt = cp.tile([128, PAIRS, S], bf16)
            if "noexp" in ablate:
                etc_ = cp.tile([128, 2, 2, W], bf16)
                nc.vector.memset(etc_[:].bitcast(mybir.dt.uint16), 0x3F00)
            if "noattn" in ablate:
                nc.vector.memset(cxt[:].bitcast(mybir.dt.uint16), 0x3F00)
            if "noproj" in ablate:
                nc.vector.memset(qt[:].bitcast(mybir.dt.uint16), 0x3F00)
                nc.vector.memset(kt[:].bitcast(mybir.dt.uint16), 0x3F00)
            wot = cp.tile([128, PAIRS, D], bf16)

            eT_r = eT[:].rearrange("(n p) s -> p n s", p=128)

            for _rep in range(reps):
                with (
                    tc.tile_pool(name="asb", bufs=3) as asb,
                    tc.tile_pool(name="projsb", bufs=1) as pjs,
                ):
                    scp_cm = tc.tile_pool(name="scp", bufs=2, space="PSUM")
                    scp = scp_cm.__enter__()
                    cxp_cm = tc.tile_pool(name="cxp", bufs=2, space="PSUM")
                    cxp = cxp_cm.__enter__()
                    pps_cm = tc.tile_pool(name="pps", bufs=2, space="PSUM")
                    pps = pps_cm.__enter__()

                    _DONE = object()
                    et_cache, rope_state = {}, {}

                    def consume(filler, n):
                        if filler is None:
                            return
                        for _ in range(n):
                            if next(filler, _DONE) is _DONE:
                                return

                    def proj_chunk(pair, cc, with_v, wqt, wkt, wvt,
                                   eager=False):
                        """One 512-col chunk of Q^T/K^T (+V when with_v) as
                        a generator of small PE quanta. RoPE state (t, u2)
                        accumulates across a 1024-col chunk pair; the
                        rotate-half DMAs and adds fire when the odd chunk
                        (or an eager chunk) completes."""
                        if "noproj" in ablate:
                            return
                        hh, sub = cc // 2, cc % 2
                        etk = (pair, hh)
                        if sub == 0:
                            # the eT half depends only on the column range,
                            # not the head-pair: download each 1.57MB half
                            # once and share the tile across all three
                            # pairs' projections
                            if hh not in et_cache:
                                etAB = pjs.tile([128, 6, HW_], bf16,
                                                tag=f"et{hh}", bufs=1,
                                                name=f"eA{hh}")
                                et_cache[hh] = etAB
                                c0 = HW_ * hh
                                if eager:
                                    for d in range(6):
                                        nc.sync.dma_start(
                                            etAB[:, d:d + 1, 0:512],
                                            eT_r[:, d:d + 1, c0:c0 + 512])
                                    nc.sync.dma_start(
                                        etAB[:, :, 512:HW_],
                                        eT_r[:, :, c0 + 512:c0 + HW_])
                                else:
                                    for sp in range(3):
                                        dd = slice(2 * sp, 2 * sp + 2)
                                        eng = (nc.sync if sp % 2 == 0
                                               else nc.scalar)
                                        eng.dma_start(etAB[:, dd, :],
                                                      eT_r[:, dd,
                                                           c0:c0 + HW_])
                            rope_state[etk] = {}
                            for wx in (0, 1):
                                rope_state[etk][wx] = (
                                    pjs.tile([128, HW_], bf16, tag=f"t{wx}",
                                             bufs=2, name=f"t{wx}{pair}{hh}"),
                                    pjs.tile([128, HW_], bf16, tag=f"u{wx}",
                                             bufs=2, name=f"u{wx}{pair}{hh}"),
                                    pjs.tile([128, HW_], bf16, tag=f"r{wx}",
                                             bufs=2, name=f"r{wx}{pair}{hh}"))
                        etAB = et_cache[hh]

                        def rope(wx, dst, ss, cols):
                            t_t, u2, ur = rope_state[etk][wx]
                            if eager:
                                # prologue: PE is idle and the SP queue is
                                # busy streaming eT — rotate via a PE
                                # permutation matmul instead of DMAs
                                ps2 = pps.tile([128, 512], f32, tag="ps",
                                               name=f"pr{wx}{pair}{cc}")
                                nc.tensor.matmul(ps2[:], prm[:], u2[:, ss],
                                                 start=True, stop=True)
                                nc.vector.tensor_tensor(
                                    dst[:, pair, cols], ps2[:], t_t[:, ss],
                                    OP.add)
                                return
                            for g in range(4):
                                d0, s0 = 32 * g, 32 * (g ^ 1)
                                nc.sync.dma_start(ur[d0:d0 + 32, ss],
                                                  u2[s0:s0 + 32, ss])
                            nc.gpsimd.tensor_tensor(
                                dst[:, pair, cols], t_t[:, ss],
                                ur[:, ss], OP.add)

                        cols = slice(512 * cc, 512 * cc + 512)
                        ss = slice(512 * sub, 512 * sub + 512)
                        pend_rope = []
                        for wx, wt, dst in ((0, wqt, qt), (1, wkt, kt)):
                            t_t, u2, ur = rope_state[etk][wx]
                            ps = pps.tile([128, 512], f32, tag="ps",
                                          name=f"ps{wx}{pair}{cc}")
                            for d in range(DC):
                                nc.tensor.matmul(
                                    ps[:],
                                    wt[:, d, 128 * pair:128 * pair + 128],
                                    etAB[:, d, 512 * sub:512 * sub + 512],
                                    start=(d == 0), stop=(d == DC - 1))
                                if d % 2 == 1:
                                    yield
                            nc.vector.tensor_tensor(
                                u2[:, ss], ps[:], sinN[:, cols], OP.mult)
                            nc.vector.tensor_tensor(
                                t_t[:, ss], ps[:], cosF[:, cols], OP.mult)
                            if eager:
                                # defer the perm matmul past the V chunks so
                                # its DVE dependency doesn't block the
                                # in-order PE queue
                                pend_rope.append((wx, dst, ss, cols))
                            yield
                        if sub == 1 and not eager:
                            hc = slice(HW_ * hh, HW_ * hh + HW_)
                            hs = slice(0, HW_)
                            rope(0, qt, hs, hc)
                            rope(1, kt, hs, hc)
                            yield
                        if with_v:
                            # V reuses the just-streamed eT bytes — keep it
                            # with its chunk so PE work per DMA byte stays
                            # above the stream rate
                            yield from v_chunks(4 * cc, 4 * cc + 4, etAB)
                        for args in pend_rope:
                            rope(*args)
                        if pend_rope:
                            yield

                    def v_chunks(lo, hi, etAB):
                        for i in range(lo, hi):
                            io = 128 * (i % 8)
                            sub, ioo = io // 512, io % 512
                            pv = pps.tile([128, 384], f32, tag="ps",
                                          name=f"pv{i}")
                            for d in range(DC):
                                nc.tensor.matmul(
                                    pv[:],
                                    etAB[:, d, 512 * sub + ioo:
                                         512 * sub + ioo + 128],
                                    wvt[:, d, :],
                                    start=(d == 0), stop=(d == DC - 1))
                                if d % 2 == 1 and d < 5:
                                    yield
                            # ACT is idle early in the kernel (few exps
                            # yet) while DVE carries the RoPE chain — evict
                            # V on ACT
                            nc.scalar.activation(vt[:, i, :, 0:HD], pv[:],
                                                 AF.Copy)
                            yield

                    def attn_win(pair, base, width, filler=None, warmup=0,
                                 evict_act=False):
                        """Attention for both heads of `pair` on the q-window
                        [base, base+width). `warmup` delays filler
                        consumption by that many k-chunks so fillers whose
                        inputs come from the previous window's eviction
                        don't stall the in-order PE queue."""
                        if "noattn" in ablate:
                            consume(filler, 1000)
                            return
                        nj = (base + width) // 128
                        CA = cxp.tile([HD + 1, W], f32, tag="C",
                                      name=f"CA{pair}{base}")
                        CB = cxp.tile([HD + 1, W], f32, tag="C",
                                      name=f"CB{pair}{base}")

                        nfull = base // 128

                        def emit_pv(j, et_, qlo):
                            off = qlo - base
                            wj = width - off
                            stop = j == nj - 1
                            for hx, C in ((0, CA), (1, CB)):
                                nc.tensor.matmul(
                                    C[:, off:off + wj],
                                    vt[:, j, 2 * pair + hx, :],
                                    et_[:, hx, 0:wj],
                                    start=(j == 0), stop=stop)

                        pend = None
                        for j in range(nj):
                            qlo = max(base, 128 * j)
                            wj = base + width - qlo
                            kk = slice(128 * j, 128 * j + 128)
                            sc = scp.tile([128, 2, W], f32, tag="sc",
                                          name=f"sc{pair}{base}{j}")
                            diag = j >= nfull
                            # two heads' score matmuls are row-tiled (K=64
                            # stationaries at partition bases 0/64) so they
                            # occupy disjoint PE row groups and overlap
                            for hx in (0, 1):
                                b0 = 64 * hx
                                nc.tensor.matmul(
                                    sc[:, hx, 0:wj],
                                    kt[b0:b0 + 64, pair, kk],
                                    qt[b0:b0 + 64, pair, qlo:qlo + wj],
                                    start=True, stop=True)
                            if "noexp" in ablate:
                                et_ = etc_
                            else:
                                et_ = asb.tile([128, 2, W], bf16, tag="ex",
                                               bufs=3,
                                               name=f"ex{pair}{base}{j}")
                                nc.scalar.activation(
                                    et_[:, :, 0:wj], sc[:, :, 0:wj],
                                    AF.Exp, scale=0.125)
                                if diag:
                                    # zero the upper-triangle weights (the
                                    # causal mask) on DVE, off the critical
                                    # sc->exp path (PV runs one j behind)
                                    nc.vector.tensor_tensor(
                                        et_[:, :, 0:128], et_[:, :, 0:128],
                                        trm[:], OP.mult)
                            # software pipeline: PV runs one j behind so
                            # PE's in-order queue never waits on exp_j
                            if pend is not None:
                                emit_pv(*pend)
                            pend = (j, et_, qlo)
                            # slip quanta of projection/output work into
                            # the PE queue behind this step's matmuls
                            # (narrow windows have tiny attn matmuls and
                            # need a faster feed to stay busy)
                            if j >= warmup:
                                consume(filler, 2 if width <= 256 else 1)
                        emit_pv(*pend)

                        cs = slice(base, base + width)
                        rec = asb.tile([128, W], bf16, tag="rec", bufs=2,
                                       name=f"rc{pair}{base}")
                        for hx, C in ((0, CA), (1, CB)):
                            po = HD * hx
                            # evict first (frees the PSUM C ring slot
                            # fast), then scale by 1/denom in place. At the
                            # tail ACT does the copy (no exps left there).
                            if evict_act:
                                nc.scalar.activation(
                                    cxt[po:po + HD, pair, cs],
                                    C[0:HD, 0:width], AF.Copy)
                            else:
                                nc.vector.tensor_copy(
                                    cxt[po:po + HD, pair, cs],
                                    C[0:HD, 0:width])
                            rr = asb.tile([1, W], bf16, tag="rr", bufs=2,
                                          name=f"rr{pair}{base}{hx}")
                            nc.vector.reciprocal(rr[0:1, 0:width],
                                                 C[HD:HD + 1, 0:width])
                            nc.sync.dma_start(
                                rec[po:po + HD, 0:width],
                                rr[0:1, None, 0:width].to_broadcast(
                                    [1, HD, width]))
                            nc.vector.tensor_tensor(
                                cxt[po:po + HD, pair, cs],
                                cxt[po:po + HD, pair, cs],
                                rec[po:po + HD, 0:width], OP.mult)
                        consume(filler, 1000)   # drain

                    def out_emit(i, osp):
                        op_ = osp.tile([128, D], f32, tag="op", name=f"op{i}")
                        ss = slice(128 * i, 128 * i + 128)
                        for pair in range(PAIRS):
                            for c0 in range(0, D, 512):
                                cw = min(512, D - c0)
                                nc.tensor.matmul(
                                    op_[:, c0:c0 + cw],
                                    cxt[:, pair, ss],
                                    wot[:, pair, c0:c0 + cw],
                                    start=(pair == 0),
                                    stop=(pair == PAIRS - 1))
                        ot = asb.tile([128, D], bf16, tag="ot", bufs=3,
                                      name=f"ot{i}")
                        # tail: evict halves on ACT and DVE in parallel,
                        # store halves on both DMA queues
                        nc.scalar.activation(ot[:, 0:384], op_[:, 0:384],
                                             AF.Copy)
                        nc.vector.tensor_copy(ot[:, 384:D], op_[:, 384:D])
                        nc.sync.dma_start(o[ss, 0:384], ot[:, 0:384])
                        nc.scalar.dma_start(o[ss, 384:D], ot[:, 384:D])

                    def out_gen(lo, hi, osp):
                        for i in range(lo, hi):
                            op_ = osp.tile([128, D], f32, tag="op",
                                           name=f"op{i}")
                            ss = slice(128 * i, 128 * i + 128)
                            for pair in range(PAIRS):
                                for c0 in range(0, D, 512):
                                    cw = min(512, D - c0)
                                    nc.tensor.matmul(
                                        op_[:, c0:c0 + cw],
                                        cxt[:, pair, ss],
                                        wot[:, pair, c0:c0 + cw],
                                        start=(pair == 0),
                                        stop=(pair == PAIRS - 1))
                                yield
                            ot = asb.tile([128, D], bf16, tag="ot",
                                          bufs=3, name=f"ot{i}")
                            # DVE copy: its queue is shorter than ACT's exp
                            # backlog, so the PSUM ring slot frees sooner
                            nc.vector.tensor_copy(ot[:], op_[:])
                            eng = nc.sync if i % 2 == 0 else nc.scalar
                            eng.dma_start(o[ss, :], ot[:])
                            yield

                    # weights/tables on the ScalarE DMA queue so the eT
                    # stream (SP queue) starts immediately. Pair-0 weight
                    # slices and the first cos/sin half come first — they
                    # gate the eager projection and window 0.
                    wq_r = wq[:].rearrange("(n p) m -> p n m", p=128)
                    wk_r = wk[:].rearrange("(n p) m -> p n m", p=128)
                    wqt = pjs.tile([128, DC, 384], bf16)
                    for dd0 in range(0, DC, 2):
                        nc.scalar.dma_start(
                            wqt[:, dd0:dd0 + 2, 0:128],
                            wq_r[:, dd0:dd0 + 2, 0:128])
                    wkt = pjs.tile([128, DC, 384], bf16)
                    nc.scalar.dma_start(wkt[:, :, 0:128], wk_r[:, :, 0:128])
                    nc.scalar.dma_start(cosF[:, 0:512], cosF_d[:, 0:512])
                    nc.scalar.dma_start(sinN[:, 0:512], sinN_d[:, 0:512])
                    wvt = pjs.tile([128, DC, 384], bf16)
                    nc.scalar.dma_start(
                        wvt[:], wv[:].rearrange("(n p) m -> p n m", p=128))
                    nc.scalar.dma_start(trm[:], trm_d[:])
                    nc.scalar.dma_start(prm[:], prm_d[:])
                    nc.scalar.dma_start(wqt[:, :, 128:384],
                                        wq_r[:, :, 128:384])
                    nc.scalar.dma_start(wkt[:, :, 128:384],
                                        wk_r[:, :, 128:384])
                    nc.scalar.dma_start(cosF[:, 512:S], cosF_d[:, 512:S])
                    nc.scalar.dma_start(sinN[:, 512:S], sinN_d[:, 512:S])
                    # wot is not needed until the output projection — keep it
                    # off the SP queue so the first eT chunk lands immediately
                    nc.scalar.dma_start(
                        wot[:], wo[:].rearrange("(n p) m -> p n m", p=128))

                    # pipeline: attention windows are the backbone;
                    # projection and output-projection matmuls are drip-fed
                    # between attention steps. win(p, w) needs pair-p halves
                    # 0..(w>=2) only; each g(p, h) covers 1024 q-columns.
                    from itertools import chain as _chain

                    def g(pair, cc, eager=False):
                        return proj_chunk(pair, cc, pair == 0,
                                          wqt, wkt, wvt, eager=eager)

                    # warm-up: throwaway matmuls against the first
                    # weight slice while the eT stream lands, so the PE
                    # clock is ramped (and HW HAM un-throttled) before the
                    # first real projection matmuls issue
                    warm = pps.tile([128, 128], f32, tag="ps", name="warm")
                    for _ in range(24):
                        nc.tensor.matmul(warm[:], wqt[:, 0, 0:128],
                                         wqt[:, 0, 0:128],
                                         start=True, stop=True)
                    consume(g(0, 0, eager=True), 1000)
                    consume(g(0, 1, eager=True), 1000)  # feeds win(0, 0..1)
                    attn_win(0, 0, W, g(0, 2))
                    attn_win(0, W, W, g(0, 3))
                    attn_win(0, 2 * W, W, g(1, 0))
                    attn_win(0, 3 * W, W, g(1, 1))
                    attn_win(1, 0, W, g(1, 2))
                    attn_win(1, W, W, g(1, 3))
                    attn_win(1, 2 * W, W, g(2, 0))
                    attn_win(1, 3 * W, W, g(2, 1))
                    attn_win(2, 0, W, _chain(g(2, 2), g(2, 3)))
                    # projection PSUM banks are dead now — recycle for the
                    # output projection so it overlaps pair-2 attention
                    pps_cm.__exit__(None, None, None)
                    osp_cm = tc.tile_pool(name="osp", bufs=1, space="PSUM")
                    osp = osp_cm.__enter__()
                    attn_win(2, W, W, out_gen(0, 4, osp), warmup=2)
                    attn_win(2, 2 * W, W, out_gen(4, 8, osp), warmup=2)
                    # the last 512-query window runs as two 256-col halves
                    # so its output chunks drain during attention instead
                    # of in a serial tail
                    # last 512-query window as 384 + 128 halves: the
                    # final 128-col window leaves only out-chunk 15 as a
                    # serial tail, and its spilled out matmuls overlap the
                    # final eviction chain on the other engines
                    attn_win(2, 3 * W, 384, out_gen(8, 10, osp),
                             warmup=2, evict_act=True)
                    attn_win(2, 3 * W + 384, 128,
                             _chain(out_gen(10, 12, osp),
                                    out_gen(12, 15, osp)),
                             warmup=2, evict_act=True)
                    osp_cm.__exit__(None, None, None)
                    cxp_cm.__exit__(None, None, None)
                    scp_cm.__exit__(None, None, None)
                    # tail: final chunk with a deep PSUM ring
                    osp2_cm = tc.tile_pool(name="osp2", bufs=3, space="PSUM")
                    osp2 = osp2_cm.__enter__()
                    out_emit(15, osp2)
                    osp2_cm.__exit__(None, None, None)
                    if dbg:
                        nc.sync.dma_start(qtd[:], qt[:])
                        nc.sync.dma_start(ktd[:], kt[:])
                        nc.sync.dma_start(vtd[:], vt[:])
                        nc.sync.dma_start(cxtd[:], cxt[:])

    nc.compile()
    return nc


def _get_program(reps=1, ablate=()):
    key = (reps, tuple(ablate))
    if key not in _CACHE:
        _CACHE[key] = _build_program(reps, ablate=ablate)
    return _CACHE[key]


def make_in_maps(embeds, Wq, Wk, Wv, Wo):
    import ml_dtypes
    bf16 = ml_dtypes.bfloat16
    f8 = ml_dtypes.float8_e4m3
    cosF, sinN = _rope_tables()
    cosF, sinN = cosF.astype(bf16), sinN.astype(bf16)
    prm = np.zeros((128, 128), np.float32)
    prm[np.arange(128), np.arange(128) ^ 32] = 1.0
    prm = prm.astype(bf16)
    trm1 = (np.arange(128)[None, :] >= np.arange(128)[:, None])
    trm = np.ascontiguousarray(
        np.broadcast_to(trm1[:, None, :], (128, 2, 128))).astype(bf16)
    eTs = [np.ascontiguousarray(embeds[b].T).astype(bf16) for b in range(B)]
    in_maps = []
    for c in range(N_CORES):
        b, hg = c // 2, c % 2
        hs = slice(hg * 384, hg * 384 + 384)
        in_maps.append({
            "eT": eTs[b],
            "wq": np.ascontiguousarray(Wq[hs].T).astype(bf16),
            "wk": np.ascontiguousarray(Wk[hs].T).astype(bf16),
            "wv": np.ascontiguousarray(Wv[hs].T).astype(bf16),
            "wo": np.ascontiguousarray(Wo[:, hs].T).astype(bf16),
            "cosF": cosF, "sinN": sinN, "trm": trm, "prm": prm,
        })
    return in_maps


def kernel(embeds, Wq, Wk, Wv, Wo, bo):
    from concourse.bass_utils import run_bass_kernel_spmd

    embeds = np.asarray(embeds, np.float32)
    Wq = np.asarray(Wq, np.float32)
    Wk = np.asarray(Wk, np.float32)
    Wv = np.asarray(Wv, np.float32)
    Wo = np.asarray(Wo, np.float32)
    bo = np.asarray(bo, np.float32)

    nc = _get_program()
    in_maps = make_in_maps(embeds, Wq, Wk, Wv, Wo)
    res = run_bass_kernel_spmd(nc, in_maps, list(range(N_CORES))).results
    out = np.empty((B, S, D), np.float32)
    for b in range(B):
        out[b] = (res[2 * b]["o"].astype(np.float32)
                  + res[2 * b + 1]["o"].astype(np.float32) + bo)
    return out
